# revision 5
# baseline (speedup 1.0000x reference)
"""Soft Needleman-Wunsch forward (logsumexp semiring) on Trainium2, 8 cores.

Algorithm: the NW recurrence
    V[i,j] = theta[i-1,j-1] + lse(A+V[i-1,j], V[i-1,j-1], A+V[i,j-1])
is LINEAR in exp space.  With the rescaling
    W[i,j] = exp(V[i,j] - alpha_b*(i+j) + shift)
the recurrence becomes
    W[i,j] = (z[j] + W[i,j-1]) * c[i,j]
    c = exp(theta + A - alpha_b)
    z[j] = W[i-1,j] + r[i,j] * W[i-1,j-1],   r = exp(-A - alpha_b)
so each DP row is ONE hardware linear-recurrence scan
(tensor_tensor_scan, op0=add, op1=mult) plus two elementwise DVE ops.

alpha_b is a PER-BATCH drift estimate (linear model on cheap batch
moments, coefficients fit offline on the randn input distribution);
shift = -30 nats centers the f32 dynamic range. A host-side retry loop
nudges alpha for any batch that still over/underflows.

Per core (16 batch elements): columns split into 8 strips of 64;
partition p = 16*s + b.  Strips run in a pipelined wavefront, strip s
lagging strip s-1 by LAG rows; the strip-boundary handoff (one f32 per
row per strip) is batched into one SBUF->SBUF DMA per GW rows.  Row i
of strip s executes at global step t = i + LAG*s.

Inputs are pre-summed (sA = theta + A) and RELAID OUT ON HOST into the
consumption order [strip, batch, row, col64] in bf16, so each input
window is ONE DMA per tensor with 4KB-contiguous per-partition
descriptors (the strided f32 gather was 256B-descriptor-bound and
stalled the wavefront).
"""
import numpy as np

# ---- constants (self-contained; hardcoded for the 128x512x512 problem) ----
B_FULL, N, M = 128, 512, 512
NCORES = 8
B = B_FULL // NCORES      # 16 batch elements per core
S = 8                     # column strips
WJ = M // S               # 64 = strip width
LAG = 12                  # inter-strip row lag (steps)
GW = 4                    # handoff batching window (steps)
T_TOTAL = N + (S - 1) * LAG   # total steps
WRING = 64                # W row ring depth (slots)
WPITCH = WJ + 1           # 65 floats per slot per partition
IRING = 64                # input ring depth (steps)
G = 32                    # input DMA window (steps)
GP = 8                    # prep (exp) grouping (steps)
PF_IN = 32                # input DMA prefetch distance (steps)
PF_PREP = 16              # exp prep prefetch distance
PF_GUARD = 4              # handoff DMA prefetch distance
PAD_LO = LAG * (S - 1)    # host row padding below row 0 (strip-7 lead-in)
PAD_HI = LAG * (S - 1)    # host row padding above row N (strip-0 tail)
NROWS = N + PAD_LO + PAD_HI
SHIFT = -30.0             # f32 dynamic-range centering (nats)
# alpha_b = coef . [1, mean(th), mean(A), var(th), var(A), cov(th,A)]
ALPHA_COEF = np.array([
    1.3802626430238012, 2.2401929737641173, 0.05134738286474591,
    -0.3900552663145703, 0.5349339712247725, 0.44477178104163556,
], dtype=np.float64)

_CACHE = {}


def _build_nc():
    import concourse.bass as bass
    import concourse.bacc as bacc
    import concourse.mybir as mybir
    import concourse.tile as tile
    from contextlib import ExitStack

    f32 = mybir.dt.float32
    bf16 = mybir.dt.bfloat16
    Alu = mybir.AluOpType
    ActFn = mybir.ActivationFunctionType

    nc = bacc.Bacc("TRN2", target_bir_lowering=False)
    # host-relaid inputs: [strip, batch, padded row, col] bf16
    sa_d = nc.dram_tensor("sA", [S, B, NROWS, WJ], bf16, kind="ExternalInput")
    a_d = nc.dram_tensor("A", [S, B, NROWS, WJ], bf16, kind="ExternalInput")
    # per-batch rescaling tables
    w0_d = nc.dram_tensor("w0", [128, WPITCH], f32, kind="ExternalInput")
    bt_d = nc.dram_tensor("bt", [B, N], f32, kind="ExternalInput")
    bias_d = nc.dram_tensor("bias", [128, 1], f32, kind="ExternalInput")
    out_d = nc.dram_tensor("out", [B, 1], f32, kind="ExternalOutput")

    # persistent SBUF ring tensors (fixed addresses -> simple strided APs)
    w_ring = nc.alloc_sbuf_tensor("w_ring", [128, WPITCH * WRING], f32)
    s_ring = nc.alloc_sbuf_tensor("s_ring", [128, WJ * IRING], bf16)
    a_ring = nc.alloc_sbuf_tensor("a_ring", [128, WJ * IRING], bf16)
    c_ring = nc.alloc_sbuf_tensor("c_ring", [128, WJ * IRING], f32)
    r_ring = nc.alloc_sbuf_tensor("r_ring", [128, WJ * IRING], f32)
    bias_t = nc.alloc_sbuf_tensor("bias_t", [128, 1], f32)

    PW = WPITCH * WRING   # w_ring partition pitch (elements)
    PI = WJ * IRING       # input ring partition pitch

    def wr_ap(p0, np_, foff, dims):
        return bass.AP(w_ring, p0 * PW + foff, [[PW, np_]] + dims)

    def ir_ap(ring, p0, np_, foff, dims):
        return bass.AP(ring, p0 * PI + foff, [[PI, np_]] + dims)

    def wslot(t):
        return (t % WRING) * WPITCH

    def islot(t):
        # input slot keyed to t-1 so G-windows [Gk+1, Gk+G+1) never wrap
        return ((t - 1) % IRING) * WJ

    with tile.TileContext(nc) as tc, ExitStack() as ctx:
        tmp_pool = ctx.enter_context(tc.tile_pool(name="tmp", bufs=8))

        # ---- one-time init ----
        nc.sync.dma_start(bias_t.ap(), bass.AP(bias_d, 0, [[1, 128], [1, 1]]))
        nc.vector.memset(w_ring.ap(), 0.0)

        def emit_input_window(t0):
            """One DMA per tensor: all 8 strips' rows for steps [t0,t0+G).

            Strip s consumes row (t - LAG*s - 1) at step t; the per-strip
            row offset is folded into the strip stride (rows are padded on
            host so every index is in-bounds; pad rows are zeros)."""
            t0 = max(t0, 1)
            tend = min(t0 + G, T_TOTAL + 1)
            if t0 >= tend:
                return
            nt = tend - t0
            # row consumed at step t for strip s: t - LAG*s - 1
            # padded index = PAD_LO + t - LAG*s - 1; LAG*s folds into the
            # strip stride below
            off = (PAD_LO + t0 - 1) * WJ
            sdims = [[B * NROWS * WJ - LAG * WJ, S], [NROWS * WJ, B],
                     [1, nt * WJ]]
            for dram, ring in ((sa_d, s_ring), (a_d, a_ring)):
                src = bass.AP(dram, off, sdims)
                dst = ir_ap(ring, 0, 128, islot(t0), [[1, nt * WJ]])
                nc.sync.dma_start(dst, src)

        def emit_prep_window(t0):
            """c = exp(sA - al_b); r = exp(-A - al_b)."""
            t0 = max(t0, 1)
            tend = min(t0 + GP, T_TOTAL + 1)
            if t0 >= tend:
                return
            nt = tend - t0
            s_s = ir_ap(s_ring, 0, 128, islot(t0), [[1, WJ * nt]])
            a_s = ir_ap(a_ring, 0, 128, islot(t0), [[1, WJ * nt]])
            c_s = ir_ap(c_ring, 0, 128, islot(t0), [[1, WJ * nt]])
            r_s = ir_ap(r_ring, 0, 128, islot(t0), [[1, WJ * nt]])
            nc.scalar.activation(c_s, s_s, ActFn.Exp, bias=bias_t.ap())
            nc.scalar.activation(r_s, a_s, ActFn.Exp, bias=bias_t.ap(), scale=-1.0)

        def emit_guard_window(w0):
            """Handoff: dst slots [w0, w0+GW) col0 p16..128 <- src slots
            [w0-LAG, ..) col64 p0..112; plus strip-0 boundary from bt."""
            if w0 >= T_TOTAL:
                return
            gw = min(GW, T_TOTAL + 1 - w0)
            with nc.allow_non_contiguous_dma(reason="strip handoff scatter"):
                src = wr_ap(0, 112, wslot(w0 - LAG) + WJ, [[WPITCH, gw]])
                dst = wr_ap(16, 112, wslot(w0) + 0, [[WPITCH, gw]])
                nc.scalar.dma_start(dst, src)
                # strip 0 boundary: rows i = t in [max(w0,1), min(w0+gw, N+1))
                ta = max(w0, 1)
                tb = min(w0 + gw, N + 1)
                if tb > ta:
                    bsrc = bass.AP(bt_d, ta - 1, [[N, B], [1, tb - ta]])
                    bdst = wr_ap(0, 16, wslot(ta) + 0, [[WPITCH, tb - ta]])
                    nc.scalar.dma_start(bdst, bsrc)

        def emit_prefill(s):
            """Row-0 slot content for strip s into slot (LAG*s)."""
            src = bass.AP(w0_d, 16 * s * WPITCH, [[WPITCH, 16], [1, WPITCH]])
            dst = wr_ap(16 * s, 16, wslot(LAG * s), [[1, WPITCH]])
            nc.sync.dma_start(dst, src)

        # ---- prologue ----
        emit_prefill(0)
        emit_input_window(1)
        for t0 in range(1, PF_PREP + 1, GP):
            emit_prep_window(t0)
        for w0 in range(0, PF_GUARD + GW, GW):
            emit_guard_window(w0)

        # ---- main unrolled loop ----
        for t in range(1, T_TOTAL + 1):
            if t % G == 1:
                emit_input_window(t + PF_IN)
            if t % GP == 1:
                emit_prep_window(t + PF_PREP)
            if t % GW == 0:
                emit_guard_window(t + PF_GUARD)

            wp = wslot(t - 1)        # previous row slot
            wc = wslot(t)            # current row slot
            ci = islot(t)
            m_t = tmp_pool.tile([128, WJ], f32, tag="m")
            z_t = tmp_pool.tile([128, WJ], f32, tag="z")
            # m = r * Wprev[j-1]
            nc.vector.tensor_tensor(
                m_t[:], ir_ap(r_ring, 0, 128, ci, [[1, WJ]]),
                wr_ap(0, 128, wp + 0, [[1, WJ]]), Alu.mult)
            # z = m + Wprev[j]
            nc.vector.tensor_tensor(
                z_t[:], m_t[:], wr_ap(0, 128, wp + 1, [[1, WJ]]), Alu.add)
            # W[:, j] = (z[j] + state) * c[j],  state0 = col0 boundary
            nc.vector.tensor_tensor_scan(
                wr_ap(0, 128, wc + 1, [[1, WJ]]),
                z_t[:],
                ir_ap(c_ring, 0, 128, ci, [[1, WJ]]),
                wr_ap(0, 128, wc + 0, [[1, 1]]),
                op0=Alu.add, op1=Alu.mult)

            if t % LAG == 0 and t // LAG < S:
                emit_prefill(t // LAG)

        # ---- finale: ship raw W[N, M] (strip 7, partitions 112..128) ----
        fin = wslot(T_TOTAL) + WJ
        nc.sync.dma_start(
            bass.AP(out_d, 0, [[1, B], [1, 1]]), wr_ap(112, B, fin, [[1, 1]]))

    nc.finalize()
    return nc


def _get_nc():
    if "nc" not in _CACHE:
        _CACHE["nc"] = _build_nc()
    return _CACHE["nc"]


def _alpha_estimate(theta, A):
    """Per-batch drift rate from cheap moments (float64)."""
    Bf = theta.shape[0]
    t = theta.reshape(Bf, -1).astype(np.float64)
    a = A.reshape(Bf, -1).astype(np.float64)
    mt = t.mean(1)
    ma = a.mean(1)
    vt = t.var(1)
    va = a.var(1)
    cov = (t * a).mean(1) - mt * ma
    X = np.stack([np.ones(Bf), mt, ma, vt, va, cov], 1)
    return X @ ALPHA_COEF


def _tables(alpha):
    """Per-core rescaling tables for a [B_FULL] alpha vector."""
    sv = np.arange(128) // B          # strip index per partition
    fv = np.arange(WPITCH)
    iv = np.arange(1, N + 1)
    w0s, bts, biases = [], [], []
    for c in range(NCORES):
        al = alpha[c * B:(c + 1) * B]           # [16]
        alp = np.tile(al, S)                    # [128] partition p = 16s+b
        w0 = np.exp(-alp[:, None] * (WJ * sv[:, None] + fv[None, :])
                    + SHIFT).astype(np.float32)
        bt = np.exp(-al[:, None] * iv[None, :] + SHIFT).astype(np.float32)
        bias = (-alp[:, None]).astype(np.float32)
        w0s.append(w0)
        bts.append(bt)
        biases.append(np.ascontiguousarray(bias))
    return w0s, bts, biases


def _relayout(x16):
    """[B_FULL, N, M] bf16 -> per-core [S, B, NROWS, WJ] bf16, row-padded."""
    import ml_dtypes
    out = np.zeros((NCORES, S, B, NROWS, WJ), dtype=ml_dtypes.bfloat16)
    # [8c,16b,512,8s,64] -> [c, s, b, row, col]
    v = x16.reshape(NCORES, B, N, S, WJ).transpose(0, 3, 1, 2, 4)
    out[:, :, :, PAD_LO:PAD_LO + N, :] = v
    return out


def _build_in_maps(theta, A, alpha=None):
    import ml_dtypes

    theta = np.asarray(theta, dtype=np.float32)
    A = np.asarray(A, dtype=np.float32)
    bf = ml_dtypes.bfloat16
    sa_l = _relayout((theta + A).astype(bf))
    a_l = _relayout(A.astype(bf))
    if alpha is None:
        alpha = _alpha_estimate(theta, A)
    return sa_l, a_l, alpha


def _in_maps_for(sa_l, a_l, alpha):
    w0s, bts, biases = _tables(alpha)
    return [
        {"sA": sa_l[c], "A": a_l[c], "w0": w0s[c], "bt": bts[c],
         "bias": biases[c]}
        for c in range(NCORES)
    ]


def kernel(theta, A):
    from concourse.bass_utils import run_bass_kernel_spmd

    nc = _get_nc()
    sa_l, a_l, alpha = _build_in_maps(theta, A)

    for _attempt in range(4):
        in_maps = _in_maps_for(sa_l, a_l, alpha)
        res = run_bass_kernel_spmd(nc, in_maps, core_ids=list(range(NCORES)))
        w = np.concatenate([r["out"].reshape(B) for r in res.results])
        w = w.astype(np.float64)
        bad_hi = ~np.isfinite(w)          # overflow -> inf/nan
        bad_lo = np.isfinite(w) & (w <= 0)  # total underflow -> 0
        if not (bad_hi.any() or bad_lo.any()):
            break
        alpha = alpha + 40.0 / 1024.0 * bad_hi - 40.0 / 1024.0 * bad_lo
    v = np.log(w) + alpha * (N + M) - SHIFT
    return v.astype(np.float32)
